# revision 8
# baseline (speedup 1.0000x reference)
"""VQ codebook (DiscreteBottleneck) Trainium2 kernel.

Problem: slot_embeddings [64, 256, 1024] f32, codebook [4096, 1024] f32.
Returns (quantized_st [64,256,1024] f32, codes [64,256] i32,
         probs [64,256,4096] f32, vq_loss f32 scalar).

Strategy: data-parallel over batch across 8 NeuronCores (2048 flat rows
per core). Per core:
  score[n, c] = 2 * x_n . c_c - ||c_c||^2   (= -squared-distance + ||x||^2)
  codes = argmax_c score        (same as argmin of distance)
  probs = softmax(score)        (softmax is invariant to the per-row shift)
  quantized = codebook[codes]   (indirect-DMA row gather)
  quantized_st = x + (q - x);  vq_loss partial = sum((q - x)^2)

The 2048x4096x1024 contraction runs on the PE as a 3-pass split-bf16
matmul (x = xh + xl, c = ch + cl in bf16; xc ~= xh.ch + xh.cl + xl.ch).
Products of bf16 pairs are exact in the fp32 PSUM accumulator, so this
matches fp32-matmul argmin decisions (verified == fp64 argmin on the
problem's inputs) at 3 PE cycles/column instead of fp32's 4.
Host pre-computes the bf16 hi/lo splits of x and of 2*C^T, plus ||c||^2
replicated across partitions; host does the final loss reduction (the
"all-reduce").
"""

import numpy as np

N_CORES = 8
BATCH, K, D = 64, 256, 1024
CB = 4096
N_TOTAL = BATCH * K          # 16384
N_CORE = N_TOTAL // N_CORES  # 2048
N_BLOCKS = N_CORE // 128     # 16
C_TILE = 512
N_CTILES = CB // C_TILE      # 8
N_DBLKS = D // 128           # 8
BETA = 0.25

_compiled = {}


def _build_bass():
    import concourse.bass as bass
    import concourse.bacc as bacc
    import concourse.mybir as mybir
    import concourse.tile as tile
    from concourse.masks import make_identity

    f32 = mybir.dt.float32
    bf16 = mybir.dt.bfloat16
    u32 = mybir.dt.uint32
    i32 = mybir.dt.int32
    Alu = mybir.AluOpType
    Act = mybir.ActivationFunctionType

    nc = bacc.Bacc("TRN2", target_bir_lowering=False, debug=False)
    x_in = nc.dram_tensor("x", [N_CORE, D], f32, kind="ExternalInput")
    xh_in = nc.dram_tensor("xh", [N_CORE, D], bf16, kind="ExternalInput")
    xl_in = nc.dram_tensor("xl", [N_CORE, D], bf16, kind="ExternalInput")
    cth_in = nc.dram_tensor("cth", [D, CB], bf16, kind="ExternalInput")  # hi(2*C^T)
    ctl_in = nc.dram_tensor("ctl", [D, CB], bf16, kind="ExternalInput")  # lo(2*C^T)
    sqc_in = nc.dram_tensor("sqc", [128, CB], f32, kind="ExternalInput")
    cb_in = nc.dram_tensor("cb", [CB, D], f32, kind="ExternalInput")
    probs_out = nc.dram_tensor("probs", [N_CORE, CB], f32, kind="ExternalOutput")
    qst_out = nc.dram_tensor("qst", [N_CORE, D], f32, kind="ExternalOutput")
    codes_out = nc.dram_tensor("codes", [N_CORE, 1], i32, kind="ExternalOutput")
    mse_out = nc.dram_tensor("msepart", [128, 1], f32, kind="ExternalOutput")

    cth_view = cth_in.rearrange("(t p) c -> t p c", p=128)
    ctl_view = ctl_in.rearrange("(t p) c -> t p c", p=128)

    with tile.TileContext(nc) as tc:
        with (
            tc.tile_pool(name="const", bufs=1) as const_pool,
            tc.tile_pool(name="ct", bufs=1) as ct_pool,
            tc.tile_pool(name="score", bufs=2) as score_pool,
            tc.tile_pool(name="x", bufs=2) as x_pool,
            tc.tile_pool(name="xhl", bufs=1) as xhl_pool,
            tc.tile_pool(name="xt", bufs=2) as xt_pool,
            tc.tile_pool(name="q", bufs=2) as q_pool,
            tc.tile_pool(name="small", bufs=2) as small_pool,
            tc.tile_pool(name="acc", bufs=1) as acc_pool,
            tc.tile_pool(name="psmm", bufs=4, space="PSUM") as psmm_pool,
            tc.tile_pool(name="pstr", bufs=2, space="PSUM") as pstr_pool,
        ):
            ident = const_pool.tile([128, 128], bf16, tag="ident")
            make_identity(nc, ident[:])

            sqc_sb = const_pool.tile([128, CB], f32, tag="sqc")
            nc.sync.dma_start(sqc_sb[:], sqc_in[:])

            ct_tiles = []  # [(hi, lo)] per d-block
            for t in range(N_DBLKS):
                cth_t = ct_pool.tile([128, CB], bf16, tag=f"cth{t}")
                nc.sync.dma_start(cth_t[:], cth_view[t])
                ctl_t = ct_pool.tile([128, CB], bf16, tag=f"ctl{t}")
                nc.sync.dma_start(ctl_t[:], ctl_view[t])
                ct_tiles.append((cth_t, ctl_t))

            mse_cols = acc_pool.tile([128, N_BLOCKS], f32, tag="msecols")

            for blk in range(N_BLOCKS):
                row0 = blk * 128
                x_t = x_pool.tile([128, D], f32, tag="x")
                nc.sync.dma_start(x_t[:], x_in[row0 : row0 + 128, :])
                xh_t = xhl_pool.tile([128, D], bf16, tag="xh")
                nc.sync.dma_start(xh_t[:], xh_in[row0 : row0 + 128, :])
                xl_t = xhl_pool.tile([128, D], bf16, tag="xl")
                nc.sync.dma_start(xl_t[:], xl_in[row0 : row0 + 128, :])

                # Transpose x block (hi and lo): xt[:, d*128+n] = x[n, d*128+p]
                xth_t = xt_pool.tile([128, D], bf16, tag="xth")
                xtl_t = xt_pool.tile([128, D], bf16, tag="xtl")
                for db in range(N_DBLKS):
                    dsl = slice(db * 128, (db + 1) * 128)
                    ps_h = pstr_pool.tile([128, 128], bf16, tag="pstrh")
                    nc.tensor.transpose(ps_h[:], xh_t[:, dsl], ident[:])
                    nc.scalar.copy(xth_t[:, dsl], ps_h[:])
                    ps_l = pstr_pool.tile([128, 128], bf16, tag="pstrl")
                    nc.tensor.transpose(ps_l[:], xl_t[:, dsl], ident[:])
                    nc.vector.tensor_copy(xtl_t[:, dsl], ps_l[:])

                score_t = score_pool.tile([128, CB], f32, tag="score")
                for ci in range(N_CTILES):
                    csl = slice(ci * C_TILE, (ci + 1) * C_TILE)
                    ps = psmm_pool.tile([128, C_TILE], f32, tag="psmm")
                    n_mm = 3 * N_DBLKS
                    k = 0
                    for db in range(N_DBLKS):
                        dsl = slice(db * 128, (db + 1) * 128)
                        cth_t, ctl_t = ct_tiles[db]
                        for lhsT, rhs in (
                            (xth_t[:, dsl], cth_t[:, csl]),
                            (xth_t[:, dsl], ctl_t[:, csl]),
                            (xtl_t[:, dsl], cth_t[:, csl]),
                        ):
                            nc.tensor.matmul(
                                ps[:], lhsT=lhsT, rhs=rhs,
                                start=(k == 0), stop=(k == n_mm - 1),
                            )
                            k += 1
                    # score = 2xc - ||c||^2  (psum - sqc), PSUM -> SBUF
                    nc.vector.tensor_tensor(
                        out=score_t[:, csl], in0=ps[:], in1=sqc_sb[:, csl],
                        op=Alu.subtract,
                    )

                max8 = small_pool.tile([128, 8], f32, tag="max8")
                idx8 = small_pool.tile([128, 8], u32, tag="idx8")
                nc.vector.max(max8[:], score_t[:])
                nc.vector.max_index(idx8[:], max8[:], score_t[:])

                negmax = small_pool.tile([128, 1], f32, tag="negmax")
                nc.vector.tensor_scalar_mul(negmax[:], max8[:, 0:1], -1.0)

                # exp in place over score; row sum into sumexp
                sumexp = small_pool.tile([128, 1], f32, tag="sumexp")
                nc.scalar.activation(
                    score_t[:], score_t[:], Act.Exp,
                    bias=negmax[:, 0:1], scale=1.0, accum_out=sumexp[:, 0:1],
                )
                rcp = small_pool.tile([128, 1], f32, tag="rcp")
                nc.vector.reciprocal(rcp[:], sumexp[:])
                # probs = exp * (1/sum), in place; then DMA out
                nc.vector.tensor_scalar_mul(score_t[:], score_t[:], rcp[:, 0:1])
                nc.scalar.dma_start(probs_out[row0 : row0 + 128, :], score_t[:])

                # codes out (uint32 -> int32 copy)
                codes_t = small_pool.tile([128, 1], i32, tag="codes")
                nc.vector.tensor_copy(codes_t[:], idx8[:, 0:1])
                nc.scalar.dma_start(codes_out[row0 : row0 + 128, :], codes_t[:])

                # gather codebook rows
                q_t = q_pool.tile([128, D], f32, tag="q")
                nc.gpsimd.indirect_dma_start(
                    out=q_t[:],
                    out_offset=None,
                    in_=cb_in[:, :],
                    in_offset=bass.IndirectOffsetOnAxis(ap=idx8[:, 0:1], axis=0),
                )
                # diff = q - x (in place in q); qst = x + diff (in place in x)
                nc.vector.tensor_tensor(
                    out=q_t[:], in0=q_t[:], in1=x_t[:], op=Alu.subtract
                )
                nc.vector.tensor_tensor(
                    out=x_t[:], in0=x_t[:], in1=q_t[:], op=Alu.add
                )
                nc.scalar.dma_start(qst_out[row0 : row0 + 128, :], x_t[:])
                # sum(diff^2) per partition into mse_cols[:, blk]
                nc.scalar.activation(
                    q_t[:], q_t[:], Act.Square,
                    accum_out=mse_cols[:, blk : blk + 1],
                )

            mse_fin = acc_pool.tile([128, 1], f32, tag="msefin")
            nc.vector.reduce_sum(mse_fin[:], mse_cols[:], axis=mybir.AxisListType.X)
            nc.sync.dma_start(mse_out[:], mse_fin[:])

    nc.compile()
    return nc


def _get_nc():
    if "nc" not in _compiled:
        _compiled["nc"] = _build_bass()
    return _compiled["nc"]


def kernel(slot_embeddings: np.ndarray, codebook: np.ndarray):
    import ml_dtypes
    from concourse.bass_utils import run_bass_kernel_spmd

    bf16 = ml_dtypes.bfloat16
    x = np.ascontiguousarray(slot_embeddings.reshape(N_TOTAL, D), dtype=np.float32)
    cb = np.ascontiguousarray(codebook, dtype=np.float32)

    xh = x.astype(bf16)
    xl = (x - xh.astype(np.float32)).astype(bf16)
    ct2 = np.ascontiguousarray(cb.T) * np.float32(2.0)
    cth = ct2.astype(bf16)
    ctl = (ct2 - cth.astype(np.float32)).astype(bf16)
    sqc = np.sum(cb.astype(np.float64) ** 2, axis=1).astype(np.float32)
    sqc_rep = np.ascontiguousarray(np.broadcast_to(sqc[None, :], (128, CB)))

    nc = _get_nc()
    in_maps = []
    for c in range(N_CORES):
        sl = slice(c * N_CORE, (c + 1) * N_CORE)
        in_maps.append(
            {
                "x": x[sl],
                "xh": xh[sl],
                "xl": xl[sl],
                "cth": cth,
                "ctl": ctl,
                "sqc": sqc_rep,
                "cb": cb,
            }
        )
    res = run_bass_kernel_spmd(nc, in_maps, core_ids=list(range(N_CORES)))
    _compiled["last_res"] = res

    probs = np.empty((N_TOTAL, CB), dtype=np.float32)
    qst = np.empty((N_TOTAL, D), dtype=np.float32)
    codes = np.empty((N_TOTAL,), dtype=np.int32)
    sum_sq = np.float32(0.0)
    for c, r in enumerate(res.results):
        sl = slice(c * N_CORE, (c + 1) * N_CORE)
        probs[sl] = r["probs"]
        qst[sl] = r["qst"]
        codes[sl] = r["codes"][:, 0]
        sum_sq = np.float32(sum_sq + np.float32(np.sum(r["msepart"], dtype=np.float64)))

    mse = np.float32(sum_sq / np.float32(N_TOTAL * D))
    vq_loss = np.float32(mse + np.float32(BETA) * mse)

    return (
        qst.reshape(BATCH, K, D),
        codes.reshape(BATCH, K),
        probs.reshape(BATCH, K, CB),
        vq_loss,
    )
